# revision 11
# baseline (speedup 1.0000x reference)
"""Trainium2 Bass kernel for the VRP pointer-network attention step.

Shapes (hardcoded from the problem spec):
  B=1024, N=512, E=256, H=128, A=32, 8 NeuronCores, data-parallel over B.

Numerics: everything that feeds the argmax chain is fp32-exact.
  - mm1 (enc @ W1, 63% of FLOPs) runs as SIX fp16 matmuls (1 cyc/row) using a
    host-side exact hi+lo fp16 split of both operands (dropping only the
    lo*lo term, ~2^-22 relative) instead of fp32's 4 cyc/row mode.
  - the softmax-row broadcast runs as TWO fp16 matmuls (hi+lo of a_t,
    exact reconstruction in the fp32 PSUM accumulator).
  - all other matmuls use the PE's exact fp32 (4 cyc/row) mode.

Layout: H on partitions for all matmuls (enc native [E,N] layout is the mm1
rhs). Per-row [1,N] results (u_t, logits) accumulate into shared PSUM banks
via zero-padded stationary operands, so the softmax / log-softmax / argmax /
demand-update phases run batched on [32,N] tiles.
"""

from contextlib import ExitStack

import numpy as np

import concourse.bacc as bacc
import concourse.tile as tile
from concourse import mybir
from concourse.bass_utils import run_bass_kernel_spmd

B, N, E, H, A = 1024, 512, 256, 128, 32
NCORES = 8
BC = B // NCORES          # rows per core = 128
RBLK = 32                 # rows per vector-phase block
NBLK = BC // RBLK         # 4 blocks per core
GRP = 4                   # rows per tanh/u_t stacking group
NGRP = RBLK // GRP        # 8 groups per block
F32 = mybir.dt.float32
F16 = mybir.dt.float16
I32 = mybir.dt.int32
U32 = mybir.dt.uint32
U8 = mybir.dt.uint8

NEG_INF = float("-inf")


def build_kernel(nc, xt_dve_mod=3):
    """xt_dve_mod: every k-th x_t PSUM->SBUF copy goes to DVE instead of ACT
    (engine balancing); 0 disables."""
    # ---- DRAM I/O ----
    ench_d = nc.dram_tensor("ench", [BC, E, N], F16, kind="ExternalInput").ap()
    encl_d = nc.dram_tensor("encl", [BC, E, N], F16, kind="ExternalInput").ap()
    dec_d = nc.dram_tensor("dec", [BC, H], F32, kind="ExternalInput").ap()
    load_d = nc.dram_tensor("load", [BC, 1], F32, kind="ExternalInput").ap()
    dem_d = nc.dram_tensor("dem", [BC, N], F32, kind="ExternalInput").ap()
    w1h_d = nc.dram_tensor("w1h", [E, H], F16, kind="ExternalInput").ap()
    w1l_d = nc.dram_tensor("w1l", [E, H], F16, kind="ExternalInput").ap()
    w2_d = nc.dram_tensor("w2", [H, H], F32, kind="ExternalInput").ap()
    wap_d = nc.dram_tensor("wap", [GRP, H, H], F32, kind="ExternalInput").ap()
    vap_d = nc.dram_tensor("vap", [NGRP, H, H], F32, kind="ExternalInput").ap()
    vcp_d = nc.dram_tensor("vcp", [RBLK, H, H], F32, kind="ExternalInput").ap()
    sel_d = nc.dram_tensor("sel", [RBLK, H, H], F16, kind="ExternalInput").ap()
    wwt_d = nc.dram_tensor("wwt", [H, H], F32, kind="ExternalInput").ap()
    wct_d = nc.dram_tensor("wct", [H, H], F32, kind="ExternalInput").ap()
    b12_d = nc.dram_tensor("b12", [H, 1], F32, kind="ExternalInput").ap()
    ba4_d = nc.dram_tensor("ba4", [H, 1], F32, kind="ExternalInput").ap()
    bwc_d = nc.dram_tensor("bwc", [H, 1], F32, kind="ExternalInput").ap()
    ident_d = nc.dram_tensor("ident", [H, H], F32, kind="ExternalInput").ap()
    cap_d = nc.dram_tensor("cap", [1, 1], F32, kind="ExternalInput").ap()

    lp_o = nc.dram_tensor("lp_out", [BC, N], F32, kind="ExternalOutput").ap()
    ptr_o = nc.dram_tensor("ptr_out", [BC, 1], I32, kind="ExternalOutput").ap()
    nl_o = nc.dram_tensor("nl_out", [BC, 1], F32, kind="ExternalOutput").ap()
    nd_o = nc.dram_tensor("nd_out", [BC, N], F32, kind="ExternalOutput").ap()

    with tile.TileContext(nc) as tc, ExitStack() as ctx:
        const = ctx.enter_context(tc.tile_pool(name="const", bufs=1))
        encp = ctx.enter_context(tc.tile_pool(name="encp", bufs=3))
        xtp = ctx.enter_context(tc.tile_pool(name="xtp", bufs=1))
        thp = ctx.enter_context(tc.tile_pool(name="thp", bufs=2))
        wcp = ctx.enter_context(tc.tile_pool(name="wcp", bufs=2))
        scr = ctx.enter_context(tc.tile_pool(name="scr", bufs=2))
        blk = ctx.enter_context(tc.tile_pool(name="blk", bufs=1))
        outp = ctx.enter_context(tc.tile_pool(name="outp", bufs=2))

        ps_xt = ctx.enter_context(tc.tile_pool(name="ps_xt", bufs=2, space="PSUM"))
        ps_pt = ctx.enter_context(tc.tile_pool(name="ps_pt", bufs=1, space="PSUM"))
        ps_u = ctx.enter_context(tc.tile_pool(name="ps_u", bufs=1, space="PSUM"))
        ps_bc = ctx.enter_context(tc.tile_pool(name="ps_bc", bufs=2, space="PSUM"))
        ps_wc = ctx.enter_context(tc.tile_pool(name="ps_wc", bufs=1, space="PSUM"))
        ps_l = ctx.enter_context(tc.tile_pool(name="ps_l", bufs=1, space="PSUM"))

        # ---- constants into SBUF ----
        w1h_sb = const.tile([H, 2, H], F16)   # [k-part, chunk, h]
        nc.sync.dma_start(out=w1h_sb, in_=w1h_d.rearrange("(c p) h -> p c h", c=2))
        w1l_sb = const.tile([H, 2, H], F16)
        nc.sync.dma_start(out=w1l_sb, in_=w1l_d.rearrange("(c p) h -> p c h", c=2))
        w2_sb = const.tile([H, H], F32)
        nc.sync.dma_start(out=w2_sb, in_=w2_d)
        wap_sb = const.tile([H, GRP, H], F32)
        nc.sync.dma_start(out=wap_sb, in_=wap_d.rearrange("g p h -> p g h"))
        vap_sb = const.tile([H, NGRP, H], F32)
        nc.sync.dma_start(out=vap_sb, in_=vap_d.rearrange("g p h -> p g h"))
        vcp_sb = const.tile([H, RBLK, H], F32)
        nc.sync.dma_start(out=vcp_sb, in_=vcp_d.rearrange("g p h -> p g h"))
        sel_sb = const.tile([H, RBLK, H], F16)
        nc.sync.dma_start(out=sel_sb, in_=sel_d.rearrange("g p h -> p g h"))
        a_hi = const.tile([H, N], F16)
        nc.vector.memset(a_hi, 0.0)
        a_lo = const.tile([H, N], F16)
        nc.vector.memset(a_lo, 0.0)
        wwt_sb = const.tile([H, H], F32)
        nc.sync.dma_start(out=wwt_sb, in_=wwt_d)
        wct_sb = const.tile([H, H], F32)
        nc.sync.dma_start(out=wct_sb, in_=wct_d)
        b12_sb = const.tile([H, 1], F32)
        nc.sync.dma_start(out=b12_sb, in_=b12_d)
        ba4_sb = const.tile([H, 1], F32)
        nc.sync.dma_start(out=ba4_sb, in_=ba4_d)
        bwc_sb = const.tile([H, 1], F32)
        nc.sync.dma_start(out=bwc_sb, in_=bwc_d)
        ident_sb = const.tile([H, H], F32)
        nc.sync.dma_start(out=ident_sb, in_=ident_d)
        cap_sb = const.tile([RBLK, 1], F32)
        nc.sync.dma_start(out=cap_sb, in_=cap_d.to_broadcast([RBLK, 1]))
        neginf_sb = const.tile([RBLK, N], F32)
        nc.vector.memset(neginf_sb, NEG_INF)
        iota_i = const.tile([RBLK, N], I32)
        nc.gpsimd.iota(iota_i, pattern=[[1, N]], base=0, channel_multiplier=0)
        iota_f = const.tile([RBLK, N], F32)
        nc.vector.tensor_copy(out=iota_f, in_=iota_i)

        # ---- per-row bias column: bias_all[:, r] = W2.T @ dec[r] + (b1 + b2) ----
        dec_sb = const.tile([H, H], F32)     # [row, h]; BC == H == 128
        nc.sync.dma_start(out=dec_sb, in_=dec_d)
        decT_full = ps_xt.tile([H, N], F32, tag="x")
        decT_ps = decT_full[:, 0:H]
        nc.tensor.transpose(decT_ps, dec_sb, ident_sb)
        decT_sb = const.tile([H, H], F32)    # [h, row]
        nc.scalar.copy(out=decT_sb, in_=decT_ps)
        bias_full = ps_xt.tile([H, N], F32, tag="x")
        bias_ps = bias_full[:, 0:H]
        nc.tensor.matmul(bias_ps, lhsT=w2_sb, rhs=decT_sb, start=True, stop=True)
        bias_all = const.tile([H, H], F32)   # [h, row]
        nc.scalar.activation(out=bias_all, in_=bias_ps,
                             func=mybir.ActivationFunctionType.Identity,
                             bias=b12_sb, scale=1.0)

        nrow = 0  # global row counter for engine-balancing the x_t copies

        # ---- main loop over blocks of 32 rows ----
        for b in range(NBLK):
            r0 = b * RBLK
            u_ps = ps_u.tile([H, N], F32)
            xts = []
            # ---------- pass 1 ----------
            for g in range(NGRP):
                pt_ps = ps_pt.tile([H, N], F32)
                for j in range(GRP):
                    r = r0 + g * GRP + j
                    ench_sb = encp.tile([H, 2, N], F16, tag="eh")
                    nc.sync.dma_start(
                        out=ench_sb, in_=ench_d[r].rearrange("(c p) n -> p c n", c=2))
                    encl_sb = encp.tile([H, 2, N], F16, tag="el")
                    nc.sync.dma_start(
                        out=encl_sb, in_=encl_d[r].rearrange("(c p) n -> p c n", c=2))
                    xt_ps = ps_xt.tile([H, N], F32, tag="x")
                    nc.tensor.matmul(xt_ps, lhsT=w1h_sb[:, 0, :],
                                     rhs=ench_sb[:, 0, :], start=True, stop=False)
                    nc.tensor.matmul(xt_ps, lhsT=w1l_sb[:, 0, :],
                                     rhs=ench_sb[:, 0, :], start=False, stop=False)
                    nc.tensor.matmul(xt_ps, lhsT=w1h_sb[:, 0, :],
                                     rhs=encl_sb[:, 0, :], start=False, stop=False)
                    nc.tensor.matmul(xt_ps, lhsT=w1h_sb[:, 1, :],
                                     rhs=ench_sb[:, 1, :], start=False, stop=False)
                    nc.tensor.matmul(xt_ps, lhsT=w1l_sb[:, 1, :],
                                     rhs=ench_sb[:, 1, :], start=False, stop=False)
                    nc.tensor.matmul(xt_ps, lhsT=w1h_sb[:, 1, :],
                                     rhs=encl_sb[:, 1, :], start=False, stop=True)
                    xt_sb = xtp.tile([H, N], F32, tag=f"xt{r % RBLK}")
                    nrow += 1
                    if xt_dve_mod and nrow % xt_dve_mod == 0:
                        nc.vector.tensor_scalar_add(xt_sb, in0=xt_ps,
                                                    scalar1=bias_all[:, r:r + 1])
                    else:
                        nc.scalar.activation(out=xt_sb, in_=xt_ps,
                                             func=mybir.ActivationFunctionType.Identity,
                                             bias=bias_all[:, r:r + 1], scale=1.0)
                    xts.append(xt_sb)
                    # pre-tanh for this row lands in partitions 32j..32j+32
                    nc.tensor.matmul(pt_ps, lhsT=wap_sb[:, j, :], rhs=xt_sb,
                                     start=(j == 0), stop=(j == GRP - 1))
                th_sb = thp.tile([H, N], F32)
                nc.scalar.activation(out=th_sb, in_=pt_ps,
                                     func=mybir.ActivationFunctionType.Tanh,
                                     bias=ba4_sb, scale=1.0)
                # u_t rows 4g..4g+4 accumulate into u_ps partitions 4g..4g+4
                nc.tensor.matmul(u_ps, lhsT=vap_sb[:, g, :], rhs=th_sb,
                                 start=(g == 0), stop=(g == NGRP - 1))

            # ---------- softmax over the block ----------
            negmax = blk.tile([RBLK, 1], F32)
            nc.vector.tensor_reduce(out=negmax, in_=u_ps[0:RBLK, :],
                                    axis=mybir.AxisListType.X,
                                    op=mybir.AluOpType.max, negate=True)
            a_sb = blk.tile([RBLK, N], F32)
            sumexp = blk.tile([RBLK, 1], F32)
            nc.scalar.activation(out=a_sb, in_=u_ps[0:RBLK, :],
                                 func=mybir.ActivationFunctionType.Exp,
                                 bias=negmax, scale=1.0, accum_out=sumexp)
            rec = blk.tile([RBLK, 1], F32)
            nc.vector.reciprocal(out=rec, in_=sumexp)
            nc.vector.tensor_scalar_mul(a_sb, in0=a_sb, scalar1=rec)
            nc.vector.tensor_copy(out=a_hi[0:RBLK, :], in_=a_sb)
            nc.vector.tensor_sub(a_lo[0:RBLK, :], in0=a_sb, in1=a_hi[0:RBLK, :])

            l_ps = ps_l.tile([H, N], F32)
            # ---------- pass 2 ----------
            for g in range(NGRP):
                c4_sb = scr.tile([H, GRP], F32, tag="c4")
                for j in range(GRP):
                    rr = g * GRP + j
                    bc_ps = ps_bc.tile([H, N], F32, tag="b")
                    nc.tensor.matmul(bc_ps, lhsT=sel_sb[:, rr, :], rhs=a_hi,
                                     start=True, stop=False)
                    nc.tensor.matmul(bc_ps, lhsT=sel_sb[:, rr, :], rhs=a_lo,
                                     start=False, stop=True)
                    ttr_scr = scr.tile([H, N], F32, tag="ttr")
                    nc.vector.tensor_mul(ttr_scr, in0=xts[rr], in1=bc_ps)
                    nc.vector.tensor_reduce(out=c4_sb[:, j:j + 1], in_=ttr_scr,
                                            axis=mybir.AxisListType.X,
                                            op=mybir.AluOpType.add)
                q_full = ps_bc.tile([H, N], F32, tag="b")
                q_ps = q_full[:, 0:GRP]
                nc.tensor.matmul(q_ps, lhsT=wct_sb, rhs=c4_sb,
                                 start=True, stop=True)
                q_sb = scr.tile([H, GRP], F32, tag="q4")
                nc.scalar.activation(out=q_sb, in_=q_ps,
                                     func=mybir.ActivationFunctionType.Identity,
                                     bias=bwc_sb, scale=1.0)
                for j in range(GRP):
                    rr = g * GRP + j
                    wc_ps = ps_wc.tile([H, N], F32)
                    nc.tensor.matmul(wc_ps, lhsT=wwt_sb, rhs=xts[rr],
                                     start=True, stop=True)
                    wt_sb = wcp.tile([H, N], F32)
                    nc.scalar.activation(out=wt_sb, in_=wc_ps,
                                         func=mybir.ActivationFunctionType.Tanh,
                                         bias=q_sb[:, j:j + 1], scale=1.0)
                    # logits row rr lands in partition rr of l_ps
                    nc.tensor.matmul(l_ps, lhsT=vcp_sb[:, rr, :], rhs=wt_sb,
                                     start=(rr == 0), stop=(rr == RBLK - 1))

            # ---------- block vector phase ----------
            negmax2 = blk.tile([RBLK, 1], F32)
            nc.vector.tensor_reduce(out=negmax2, in_=l_ps[0:RBLK, :],
                                    axis=mybir.AxisListType.X,
                                    op=mybir.AluOpType.max, negate=True)
            els = blk.tile([RBLK, N], F32)
            sum2 = blk.tile([RBLK, 1], F32)
            nc.scalar.activation(out=els, in_=l_ps[0:RBLK, :],
                                 func=mybir.ActivationFunctionType.Exp,
                                 bias=negmax2, scale=1.0, accum_out=sum2)
            lnz = blk.tile([RBLK, 1], F32)
            nc.scalar.activation(out=lnz, in_=sum2,
                                 func=mybir.ActivationFunctionType.Ln,
                                 bias=0.0, scale=1.0)
            offs = blk.tile([RBLK, 1], F32)
            nc.vector.tensor_sub(offs, in0=negmax2, in1=lnz)
            lp_sb = blk.tile([RBLK, N], F32)
            nc.vector.tensor_scalar_add(lp_sb, in0=l_ps[0:RBLK, :], scalar1=offs)

            dem_sb = blk.tile([RBLK, N], F32)
            nc.sync.dma_start(out=dem_sb, in_=dem_d[r0:r0 + RBLK, :])
            load_sb = blk.tile([RBLK, 1], F32)
            nc.sync.dma_start(out=load_sb, in_=load_d[r0:r0 + RBLK, :])
            loadz = blk.tile([RBLK, 1], F32)
            nc.vector.tensor_scalar(out=loadz, in0=load_sb, scalar1=0.0,
                                    scalar2=None, op0=mybir.AluOpType.is_equal)
            mzf = blk.tile([RBLK, N], F32)
            nc.vector.tensor_scalar(out=mzf, in0=dem_sb, scalar1=0.0, scalar2=None,
                                    op0=mybir.AluOpType.is_equal)
            mz = blk.tile([RBLK, N], U8)
            nc.vector.tensor_scalar(out=mz, in0=mzf, scalar1=loadz,
                                    scalar2=None, op0=mybir.AluOpType.max)
            nc.vector.memset(mz[:, 0:1], 0)
            lpm = blk.tile([RBLK, N], F32)
            nc.vector.select(lpm, mask=mz, on_true=neginf_sb, on_false=lp_sb)
            nc.sync.dma_start(out=lp_o[r0:r0 + RBLK, :], in_=lpm)

            mx8 = blk.tile([RBLK, 8], F32)
            nc.vector.max(out=mx8, in_=lpm)
            idx8 = blk.tile([RBLK, 8], U32)
            nc.vector.max_index(out=idx8, in_max=mx8, in_values=lpm)
            ptr_f = blk.tile([RBLK, 1], F32)
            nc.vector.tensor_copy(out=ptr_f, in_=idx8[:, 0:1])
            ptr_i = outp.tile([RBLK, 1], I32, tag="ptri")
            nc.vector.tensor_copy(out=ptr_i, in_=idx8[:, 0:1])
            nc.sync.dma_start(out=ptr_o[r0:r0 + RBLK, :], in_=ptr_i)

            eq = blk.tile([RBLK, N], F32)
            nc.vector.tensor_scalar(out=eq, in0=iota_f, scalar1=ptr_f, scalar2=None,
                                    op0=mybir.AluOpType.is_equal)
            prod_scr = blk.tile([RBLK, N], F32)
            dsel = blk.tile([RBLK, 1], F32)
            nc.vector.tensor_mul(prod_scr, in0=eq, in1=dem_sb)
            nc.vector.tensor_reduce(out=dsel, in_=prod_scr,
                                    axis=mybir.AxisListType.X,
                                    op=mybir.AluOpType.add)

            ad = blk.tile([RBLK, 1], U8)
            nc.vector.tensor_scalar(out=ad, in0=ptr_f, scalar1=0.0, scalar2=None,
                                    op0=mybir.AluOpType.is_equal)
            lmd = blk.tile([RBLK, 1], F32)
            nc.vector.tensor_sub(lmd, in0=load_sb, in1=dsel)
            nl = outp.tile([RBLK, 1], F32, tag="nl")
            nc.vector.select(nl, mask=ad, on_true=cap_sb, on_false=lmd)
            nc.sync.dma_start(out=nl_o[r0:r0 + RBLK, :], in_=nl)
            ndsr = blk.tile([RBLK, 1], F32)
            nc.vector.tensor_sub(ndsr, in0=dsel, in1=nl)
            nds = blk.tile([RBLK, 1], F32)
            nc.vector.select(nds, mask=ad, on_true=dsel, on_false=ndsr)

            # new_demand = dem - eq*(dem - nds)
            s1 = blk.tile([RBLK, N], F32)
            nc.vector.scalar_tensor_tensor(
                out=s1, in0=dem_sb, scalar=nds, in1=eq,
                op0=mybir.AluOpType.subtract, op1=mybir.AluOpType.mult)
            nd_sb = outp.tile([RBLK, N], F32, tag="nd")
            nc.vector.tensor_sub(nd_sb, in0=dem_sb, in1=s1)
            nc.sync.dma_start(out=nd_o[r0:r0 + RBLK, :], in_=nd_sb)

    return nc


def _split16(x):
    hi = x.astype(np.float16)
    lo = (x - hi.astype(np.float32)).astype(np.float16)
    return hi, lo


def _build_host_inputs(inputs):
    dec = np.ascontiguousarray(np.asarray(inputs["decoder_output"], np.float32)
                               .reshape(B, H))
    enc = np.ascontiguousarray(np.asarray(inputs["encoder_outputs"], np.float32))
    load = np.ascontiguousarray(np.asarray(inputs["remaining_truck_load"],
                                           np.float32).reshape(B, 1))
    dem = np.ascontiguousarray(np.asarray(inputs["remaining_customer_demand"],
                                          np.float32))
    w1 = np.asarray(inputs["W1"], np.float32)
    w2 = np.ascontiguousarray(np.asarray(inputs["W2"], np.float32))
    wa = np.asarray(inputs["Wa"], np.float32)          # [H, A]
    va = np.asarray(inputs["Va"], np.float32).reshape(A)
    wwt = np.ascontiguousarray(np.asarray(inputs["Wwt"], np.float32))
    wct = np.ascontiguousarray(np.asarray(inputs["Wct"], np.float32))
    vc = np.asarray(inputs["Vc"], np.float32).reshape(H)
    b1 = np.asarray(inputs["b1"], np.float32)
    b2 = np.asarray(inputs["b2"], np.float32)
    ba = np.asarray(inputs["ba"], np.float32)
    bwt = np.asarray(inputs["bwt"], np.float32)
    bct = np.asarray(inputs["bct"], np.float32)
    cap = np.float32(inputs["vehicle_capacity"])

    ench, encl = _split16(enc)
    w1h, w1l = _split16(w1)

    # Wa padded: wap[j][:, 32j:32j+32] = Wa -> row j's pre-tanh in parts 32j..
    wap = np.zeros((GRP, H, H), np.float32)
    for j in range(GRP):
        wap[j, :, j * A:(j + 1) * A] = wa
    # Va block-diag padded: vap[g][32j:32j+32, 4g+j] = va
    vap = np.zeros((NGRP, H, H), np.float32)
    for g in range(NGRP):
        for j in range(GRP):
            vap[g, j * A:(j + 1) * A, g * GRP + j] = va
    # Vc padded: vcp[rr][:, rr] = vc -> logits row rr in partition rr
    vcp = np.zeros((RBLK, H, H), np.float32)
    for rr in range(RBLK):
        vcp[rr, :, rr] = vc
    # row-select: sel[rr][rr, :] = 1 -> broadcast a_sb row rr to 128 partitions
    sel = np.zeros((RBLK, H, H), np.float16)
    for rr in range(RBLK):
        sel[rr, rr, :] = 1.0

    b12 = (b1 + b2).reshape(H, 1)
    ba4 = np.tile(ba, GRP).reshape(H, 1)
    bwc = (bwt + bct).reshape(H, 1)
    ident = np.eye(H, dtype=np.float32)
    common = dict(w1h=np.ascontiguousarray(w1h), w1l=np.ascontiguousarray(w1l),
                  w2=w2, wap=wap, vap=vap, vcp=vcp, sel=sel,
                  wwt=wwt, wct=wct, b12=b12, ba4=ba4, bwc=bwc, ident=ident,
                  cap=np.array([[cap]], np.float32))
    in_maps = []
    for c in range(NCORES):
        sl = slice(c * BC, (c + 1) * BC)
        in_maps.append(dict(common,
                            ench=np.ascontiguousarray(ench[sl]),
                            encl=np.ascontiguousarray(encl[sl]),
                            dec=np.ascontiguousarray(dec[sl]),
                            load=np.ascontiguousarray(load[sl]),
                            dem=np.ascontiguousarray(dem[sl])))
    return in_maps


_CACHED_NC = None


def _get_nc():
    global _CACHED_NC
    if _CACHED_NC is None:
        nc = bacc.Bacc("TRN2", target_bir_lowering=False, debug=False)
        build_kernel(nc)
        nc.compile()
        _CACHED_NC = nc
    return _CACHED_NC


def kernel(**inputs):
    in_maps = _build_host_inputs(inputs)
    nc = _get_nc()
    res = run_bass_kernel_spmd(nc, in_maps, core_ids=list(range(NCORES)))
    lp = np.concatenate([res.results[c]["lp_out"] for c in range(NCORES)], 0)
    ptr = np.concatenate([res.results[c]["ptr_out"][:, 0] for c in range(NCORES)], 0)
    nl = np.concatenate([res.results[c]["nl_out"][:, 0] for c in range(NCORES)], 0)
    nd = np.concatenate([res.results[c]["nd_out"] for c in range(NCORES)], 0)
    return lp, ptr.astype(np.int32), nl, nd
